# revision 2
# baseline (speedup 1.0000x reference)
"""MultiHeadAttention forward on 8 TRN2 NeuronCores (Bass/Tile) — v3.

Sharding: core c <-> (batch b=c//2, head-group g=c%2, 8 heads each).
Host pre-transposes x (xT [C,T]), folds the 1/32 score scale into Wq, and
supplies the projection weights in bf16.

All matmuls run in the single (128,128) tile mode (no PE drains): q/k are
stored per-head on a full 128-partition tile, with the head's 64 dims on
the partition half the QKV matmul produced them in and zeros on the other
half, so the K=128 contraction gives exact single-head scores.

Fused pipeline per 512-token chunk s: QKV(s) (f32r), then attention span s
(S^T bf16 -> exp on ACT -> causal mask (DVE bf16) -> PV bf16 with ones row
for denominators -> reciprocal_approx_fast + partition_broadcast + mul),
with QKV of chunk s+1 spliced between rounds. Pairwise AllGather of
contributed 512-chunks (bf16) overlaps; bf16 row-parallel projection at
the end.
"""
import os

import numpy as np

DEBUG = os.environ.get("KERNEL_DEBUG", "0") == "1"

B, T, C = 4, 2048, 1024
H, HS = 16, 64
HPC = 8          # heads per core
NCORES = 8
TH = T // 2      # token half owned per core (1024)
NQ = 4           # 512-token chunks
TQ = T // NQ     # 512

_CACHE = {}


def _build_nc():
    import concourse.bass as bass
    import concourse.mybir as mybir
    import concourse.tile as tile
    from concourse import bacc
    from concourse.bass import ds

    F32 = mybir.dt.float32
    F32R = mybir.dt.float32r
    BF16 = mybir.dt.bfloat16
    AF = mybir.ActivationFunctionType
    PAIRS = [[0, 1], [2, 3], [4, 5], [6, 7]]

    nc = bacc.Bacc("TRN2", target_bir_lowering=False, debug=False, num_devices=NCORES)

    xt = nc.dram_tensor("xt", [C, T], F32R, kind="ExternalInput").ap()
    wq = nc.dram_tensor("wq", [C, 512], F32R, kind="ExternalInput").ap()
    wk = nc.dram_tensor("wk", [C, 512], F32R, kind="ExternalInput").ap()
    wv = nc.dram_tensor("wv", [C, 512], F32R, kind="ExternalInput").ap()
    wpo = nc.dram_tensor("wpo", [512, C], BF16, kind="ExternalInput").ap()
    wpx = nc.dram_tensor("wpx", [512, C], BF16, kind="ExternalInput").ap()
    bpr = nc.dram_tensor("bpr", [1, C], F32, kind="ExternalInput").ap()
    y = nc.dram_tensor("y", [TH, C], F32, kind="ExternalOutput").ap()

    dbg = {}
    if DEBUG:
        for nm, shp, dt in (("dbg_attn", [128, 4, T], BF16),
                            ("dbg_own", [128, 4, TH], BF16),
                            ("dbg_rem", [128, 4, TH], BF16)):
            dbg[nm] = nc.dram_tensor(nm, shp, dt, kind="ExternalOutput").ap()

    with tile.TileContext(nc) as tc:
        pid_g = nc.gpsimd.partition_id()
        g_sv = nc.gpsimd.snap(pid_g % 2, max_val=1)
        roff = nc.gpsimd.snap(128 - g_sv * 128, max_val=128)
        toff_ctr = [nc.gpsimd.snap(TH - g_sv * TH + 512 * k, max_val=TH + 512)
                    for k in range(2)]
        toff_own = [nc.gpsimd.snap(g_sv * TH + 512 * k, max_val=TH + 512)
                    for k in range(2)]

        with tc.tile_pool(name="outer", bufs=1) as outer, \
             tc.tile_pool(name="wpool", bufs=1) as wpool:
            # causal master mask, bf16: m[p, c] = 1.0 iff c - p >= 384
            master = outer.tile([128, 512], BF16)
            nc.gpsimd.memset(master[:], 1.0)
            nc.gpsimd.affine_select(
                out=master[:], in_=master[:], compare_op=mybir.AluOpType.is_ge,
                fill=0.0, base=-384, pattern=[[1, 512]], channel_multiplier=-1)

            wq_sb = wpool.tile([128, 8, 512], F32R)
            wk_sb = wpool.tile([128, 8, 512], F32R)
            wv_sb = wpool.tile([128, 8, 512], F32R)
            for m in range(4):
                nc.scalar.dma_start(
                    out=wq_sb[:, :, m * 128:(m + 1) * 128],
                    in_=wq.rearrange("(k p) n -> p k n", p=128)[:, :, m * 128:(m + 1) * 128])
                nc.scalar.dma_start(
                    out=wk_sb[:, :, m * 128:(m + 1) * 128],
                    in_=wk.rearrange("(k p) n -> p k n", p=128)[:, :, m * 128:(m + 1) * 128])
            nc.scalar.dma_start(out=wv_sb[:], in_=wv.rearrange("(k p) n -> p k n", p=128))

            # per-chunk K/V tiles; k stays compact (2 heads on 128 partitions).
            # q is stored per-head with the OTHER head's 64 partitions zeroed,
            # so the K=128 contraction k_compact^T @ q_padded gives exact
            # single-head scores in the uniform (128,128) tile mode.
            k_c = [outer.tile([128, 4, 512], BF16, name=f"k_c{i}") for i in range(NQ)]
            v_c = [outer.tile([128, 4, 8 * 65], BF16, name=f"v_c{i}") for i in range(NQ)]
            for i in range(NQ):
                nc.gpsimd.memset(
                    v_c[i][:].rearrange("p i (h e) -> p i h e", e=65)[:, :, :, 64:65],
                    1.0)
            attn_T = outer.tile([128, 4, T], BF16)   # [d(2 heads), pair, t]
            rem_sb = outer.tile([128, 4, TH], BF16)
            own_sb = outer.tile([128, 4, TH], BF16)
            wpo_sb = outer.tile([128, 4, C], BF16)
            wpx_sb = outer.tile([128, 4, C], BF16)

            with tc.tile_pool(name="lsb", bufs=1) as lsb, \
                 tc.tile_pool(name="lps", bufs=1, space="PSUM") as lps, \
                 tc.tile_pool(name="ccd", bufs=1, space="DRAM") as ccd:

                qch = {}     # chunk -> q tile [128, 8, 512] bf16 (padded layout)

                def emit_x_dma(s):
                    xch = []
                    for cb in range(8):
                        xcb = lsb.tile([128, 512], F32R, tag="x", bufs=10,
                                       name=f"x{s}_{cb}")
                        nc.sync.dma_start(
                            out=xcb[:],
                            in_=xt.rearrange("(k p) t -> p k t", p=128)[:, cb, s * 512:(s + 1) * 512])
                        xch.append(xcb)
                    return xch

                def qkv_chains(s, xch):
                    """Yield thunks, each emitting one 8-MM chain + evac."""
                    qt = lsb.tile([128, 8, 512], BF16, tag="qch", bufs=2,
                                  name=f"q{s}")
                    qch[s] = qt
                    # stale ring contents are finite bf16 except on first use;
                    # zero the unused halves so 0*x in S never meets NaN
                    if s < 2:
                        for h in range(8):
                            mb = 64 * (h % 2)
                            nc.gpsimd.memset(qt[64 - mb:128 - mb, h, :], 0.0)

                    def q_chain(m):
                        psw = lps.tile([128, 1024], F32, tag="pss", bufs=2,
                                       name=f"psq{s}{m}")
                        ps = psw[:, 0:512]
                        for cb in range(8):
                            nc.tensor.matmul(
                                ps, wq_sb[:, cb, m * 128:(m + 1) * 128],
                                xch[cb][:], start=(cb == 0), stop=(cb == 7))
                        with nc.allow_low_precision(reason="q bf16"):
                            nc.vector.tensor_copy(qt[0:64, 2 * m, :], ps[0:64])
                            nc.vector.tensor_copy(qt[64:128, 2 * m + 1, :], ps[64:128])

                    def k_chain(m):
                        psw = lps.tile([128, 1024], F32, tag="pss", bufs=2,
                                       name=f"psk{s}{m}")
                        ps = psw[:, 0:512]
                        for cb in range(8):
                            nc.tensor.matmul(
                                ps, wk_sb[:, cb, m * 128:(m + 1) * 128],
                                xch[cb][:], start=(cb == 0), stop=(cb == 7))
                        with nc.allow_low_precision(reason="k bf16"):
                            nc.vector.tensor_copy(k_c[s][:, m, :], ps)

                    def v_chain(i):
                        psw = lps.tile([128, 1024], F32, tag="pss", bufs=2,
                                       name=f"psv{s}{i}")
                        ps = psw[:, 0:512]
                        for cb in range(8):
                            nc.tensor.matmul(
                                ps, xch[cb][:, i * 128:(i + 1) * 128],
                                wv_sb[:, cb, :], start=(cb == 0), stop=(cb == 7))
                        with nc.allow_low_precision(reason="v bf16"):
                            nc.vector.tensor_copy(
                                v_c[s][:, i, :].rearrange("p (h e) -> p h e", e=65)[:, :, 0:64],
                                ps.rearrange("p (h e) -> p h e", e=64))

                    for m in range(4):
                        yield lambda m=m: q_chain(m)
                        yield lambda m=m: k_chain(m)
                    for i in range(4):
                        yield lambda i=i: v_chain(i)

                def exchange(hp, k):
                    cc_in = ccd.tile([128, 512], BF16, tag="ccin", bufs=4,
                                     name=f"ci{hp}{k}")
                    cc_out = ccd.tile([256, 512], BF16, tag="ccout", bufs=4,
                                      name=f"co{hp}{k}")
                    nc.gpsimd.dma_start(out=cc_in[:],
                                        in_=attn_T[:, hp, ds(toff_ctr[k], 512)])
                    nc.gpsimd.collective_compute(
                        "AllGather", mybir.AluOpType.bypass,
                        ins=[cc_in.opt()], outs=[cc_out.opt()],
                        replica_groups=PAIRS)
                    nc.gpsimd.dma_start(out=rem_sb[:, hp, k * 512:(k + 1) * 512],
                                        in_=cc_out[ds(roff, 128), :])
                    nc.gpsimd.dma_start(out=own_sb[:, hp, k * 512:(k + 1) * 512],
                                        in_=attn_T[:, hp, ds(toff_own[k], 512)])

                # ---------------- main fused loop over spans ----------------
                xch0 = emit_x_dma(0)
                for th in qkv_chains(0, xch0):
                    th()
                xch_next = emit_x_dma(1)

                for s in range(NQ):
                    jmax = 4 * (s + 1)
                    splice = list(qkv_chains(s + 1, xch_next)) if s + 1 < NQ else []
                    if s + 2 < NQ:
                        xch_next = emit_x_dma(s + 2)
                    if s == 2:   # prefetch projection weights
                        nc.scalar.dma_start(
                            out=wpo_sb[:], in_=wpo.rearrange("(k p) n -> p k n", p=128))
                        nc.scalar.dma_start(
                            out=wpx_sb[:], in_=wpx.rearrange("(k p) n -> p k n", p=128))
                    sp_i = 0

                    def do_splice(n):
                        nonlocal sp_i
                        for _ in range(n):
                            if sp_i < len(splice):
                                splice[sp_i]()
                                sp_i += 1

                    # two sub-blocks of 4 heads each (PSUM: 4 pso banks)
                    def qlo_of(j):
                        u_off = j - 4 * s
                        return u_off * 128 if u_off > 0 else 0

                    def pv_round(jp, heads, pso, P_t, last):
                        for h in heads:
                            for u2 in (0, 1):
                                j = 2 * jp + u2
                                qlo = qlo_of(j)
                                nc.tensor.matmul(
                                    pso[h][:, qlo:512],
                                    v_c[j // 4][:, j % 4, h * 65:h * 65 + 65],
                                    P_t[(jp, h)][:, u2 * 512 + qlo:(u2 + 1) * 512],
                                    start=(j == 0), stop=(last and u2 == 1))
                            P_t.pop((jp, h))

                    sb_order = (1, 0) if s == NQ - 1 else (0, 1)
                    for sb in sb_order:
                        heads = range(4 * sb, 4 * sb + 4)
                        pso = {}
                        for h in heads:
                            pso[h] = lps.tile([65, 512], F32, tag="pso", bufs=4,
                                              name=f"pso{s}{h}")
                        P_t = {}
                        for jp in range(jmax // 2):
                            diag = (2 * jp + 1 - 4 * s) >= 0
                            for h in heads:
                                pss = lps.tile([128, 1024], F32, tag="pss",
                                               bufs=2, name=f"pss{s}{jp}{h}")
                                for u2 in (0, 1):
                                    j = 2 * jp + u2
                                    qlo = qlo_of(j)
                                    nc.tensor.matmul(
                                        pss[:, u2 * 512 + qlo:(u2 + 1) * 512],
                                        k_c[j // 4][:, h // 2, (j % 4) * 128:(j % 4 + 1) * 128],
                                        qch[s][:, h, qlo:512],
                                        start=True, stop=True)
                                P = lsb.tile([128, 1024], BF16, tag="P",
                                             bufs=9, name=f"P{s}{jp}{h}")
                                if not diag:
                                    nc.scalar.activation(P[:], pss[:], AF.Exp)
                                else:
                                    for u2 in (0, 1):
                                        j = 2 * jp + u2
                                        qlo = qlo_of(j)
                                        nc.scalar.activation(
                                            P[:, u2 * 512 + qlo:(u2 + 1) * 512],
                                            pss[:, u2 * 512 + qlo:(u2 + 1) * 512],
                                            AF.Exp)
                                        if j - 4 * s >= 0:
                                            nc.vector.tensor_mul(
                                                P[:, u2 * 512 + qlo:u2 * 512 + qlo + 128],
                                                P[:, u2 * 512 + qlo:u2 * 512 + qlo + 128],
                                                master[:, 384:512])
                                P_t[(jp, h)] = P
                            if jp > 0:
                                pv_round(jp - 1, heads, pso, P_t, last=False)
                            do_splice(2)
                        pv_round(jmax // 2 - 1, heads, pso, P_t, last=True)
                        for h in heads:
                            hp, hh = h // 2, h % 2
                            mb = 64 * hh
                            dcp = lsb.tile([1, 512], F32, tag="zr", bufs=2,
                                           name=f"dc{s}{h}")
                            nc.vector.tensor_copy(dcp[:], pso[h][64:65, :])
                            zr = lsb.tile([1, 512], F32, tag="zr", bufs=2,
                                          name=f"zr{s}{h}")
                            nc.vector.reciprocal_approx_fast(out=zr[:], in_=dcp[:])
                            rb = lsb.tile([64, 512], F32, tag="rb", bufs=2,
                                          name=f"rb{s}{h}")
                            nc.gpsimd.partition_broadcast(rb[:], zr[:])
                            with nc.allow_low_precision(reason="attn bf16"):
                                nc.vector.tensor_mul(
                                    attn_T[mb:mb + 64, hp, s * 512:(s + 1) * 512],
                                    pso[h][0:64, :], rb[:])
                            # exchange as soon as this head-pair's span is done
                            if s >= 2 and h % 2 == 1:
                                exchange(h // 2, s - 2)
                        do_splice(3)
                    do_splice(len(splice))

                if DEBUG:
                    nc.gpsimd.dma_start(out=dbg["dbg_attn"][:], in_=attn_T[:])
                    nc.gpsimd.dma_start(out=dbg["dbg_own"][:], in_=own_sb[:])
                    nc.gpsimd.dma_start(out=dbg["dbg_rem"][:], in_=rem_sb[:])

            # ---------------- output projection ----------------
            with tc.tile_pool(name="p3sb", bufs=1) as p3sb, \
                 tc.tile_pool(name="p3ps", bufs=1, space="PSUM") as p3ps:
                bp_sb = p3sb.tile([1, C], F32)
                nc.sync.dma_start(out=bp_sb[:], in_=bpr)
                bias_sb = p3sb.tile([128, C], F32)
                nc.gpsimd.partition_broadcast(bias_sb[:], bp_sb[:])

                for i in range(TH // 128):
                    for e in range(2):
                        psy = p3ps.tile([128, 512], F32, tag="y", bufs=8)
                        for m in range(4):
                            nc.tensor.matmul(
                                psy[:], own_sb[:, m, i * 128:(i + 1) * 128],
                                wpo_sb[:, m, e * 512:(e + 1) * 512],
                                start=(m == 0), stop=False)
                        for m in range(4):
                            nc.tensor.matmul(
                                psy[:], rem_sb[:, m, i * 128:(i + 1) * 128],
                                wpx_sb[:, m, e * 512:(e + 1) * 512],
                                start=False, stop=(m == 3))
                        ysb = p3sb.tile([128, 512], F32, tag="ysb", bufs=2)
                        nc.vector.tensor_add(ysb[:], psy[:],
                                             bias_sb[:, e * 512:(e + 1) * 512])
                        nc.sync.dma_start(
                            out=y[i * 128:(i + 1) * 128, e * 512:(e + 1) * 512],
                            in_=ysb[:])

    nc.compile()
    return nc


def _get_nc():
    if "nc" not in _CACHE:
        _CACHE["nc"] = _build_nc()
    return _CACHE["nc"]


def _make_in_maps(x, Wq, Wk, Wv, Wp, bp):
    import ml_dtypes
    in_maps = []
    for c in range(NCORES):
        b, g = c // 2, c % 2
        hsel = slice(g * HPC, (g + 1) * HPC)
        wq_c = np.ascontiguousarray(
            np.transpose(Wq[hsel], (1, 0, 2)).reshape(C, HPC * HS)) * (1.0 / 32.0)
        wk_c = np.ascontiguousarray(
            np.transpose(Wk[hsel], (1, 0, 2)).reshape(C, HPC * HS))
        wv_c = np.ascontiguousarray(
            np.transpose(Wv[hsel], (1, 0, 2)).reshape(C, HPC * HS))
        in_maps.append({
            "xt": np.ascontiguousarray(x[b].T),
            "wq": np.ascontiguousarray(wq_c), "wk": wk_c, "wv": wv_c,
            "wpo": np.ascontiguousarray(Wp[g * 512:(g + 1) * 512]).astype(ml_dtypes.bfloat16),
            "wpx": np.ascontiguousarray(Wp[(1 - g) * 512:(2 - g) * 512]).astype(ml_dtypes.bfloat16),
            "bpr": bp.reshape(1, C),
        })
    return in_maps


def kernel(x, Wq, Wk, Wv, Wp, bp):
    from concourse.bass_utils import run_bass_kernel_spmd

    x = np.asarray(x, dtype=np.float32)
    Wq = np.asarray(Wq, dtype=np.float32)
    Wk = np.asarray(Wk, dtype=np.float32)
    Wv = np.asarray(Wv, dtype=np.float32)
    Wp = np.asarray(Wp, dtype=np.float32)
    bp = np.asarray(bp, dtype=np.float32)

    nc = _get_nc()
    in_maps = _make_in_maps(x, Wq, Wk, Wv, Wp, bp)
    res = run_bass_kernel_spmd(nc, in_maps, core_ids=list(range(NCORES)))
    _CACHE["last_results"] = res

    out = np.empty((B, T, C), np.float32)
    for c in range(NCORES):
        b, g = c // 2, c % 2
        out[b, g * TH:(g + 1) * TH, :] = res.results[c]["y"]
    return out
